# revision 32
# baseline (speedup 1.0000x reference)
"""Block-circulant layer (p=q=64, bs=64) on 8 TRN2 NeuronCores.

y = x @ W_dense where W_dense is block-circulant, computed via the rfft
factorization as three dense matmul stages per core (data-parallel over rows):

  S1: per input-block pair jp: X1t[b, j*64+s] = x_t[j*64+m, b].T @ C2   (hc rfft)
  S2: per frequency f:        Y2t[b, f*128+iri] = Xcat_f.T @ Wcat_f    (complex mul)
  S3: per output-block pair:  y[b, ip*128+nn] = Ycat2_ip.T @ D2        (hc irfft)

"hc" = halfcomplex packing [Xr_0, Xr_1..31, Xr_32, Xi_1..31] -> exactly 64 real
slots per 64-wide block, so every stationary/moving operand is a clean 128x128.
Between stages, PE transposes (Sh1b/Sh2b) regroup j-major -> f-major -> i-major.
Stage outputs land directly in the layout the next stage consumes (the data is
the *stationary* operand of each matmul, so outputs come out batch-major).

Host side: x is transposed once (x_t [4096, 8192]) so no on-device input
transpose pass is needed; DFT/weight matrices are precomputed.
"""
import os
import numpy as np

N_CORES = 8
BS = 64
NBLK = 64              # p = q = 64 blocks
IN_F = NBLK * BS       # 4096
DT_NAME = os.environ.get("KERNEL_DTYPE", "fp16")
LDW_OPT = os.environ.get("KERNEL_LDW_OPT", "0") == "1"
SKEW = os.environ.get("KERNEL_SKEW", "1") == "1"
# Transposes as regular-mode matmuls (counts as PE-busy for HAM, so the
# 2.4 GHz clock stays engaged; costs the same at 16-bit dtypes).
REGT = os.environ.get("KERNEL_REGT", "0") == "1"
# Sh2b via DMA xbar transpose instead of the tensor engine.
DMAT = os.environ.get("KERNEL_DMAT", "0") == "1"

_CACHE = {}


def _patch_ldw_opt():
    """Flip walrus's --enable-ldw-opt to true for this process."""
    import concourse.bass_utils as bu

    if getattr(bu, "_ldw_patched", False):
        return
    orig = bu.run_command

    def patched(argv, **kw):
        argv = ["--enable-ldw-opt=true" if a == "--enable-ldw-opt=false" else a
                for a in argv]
        return orig(argv, **kw)

    bu.run_command = patched
    bu._ldw_patched = True


# ---------------------------------------------------------------- host matrices
def _hc_dft_matrices():
    m = np.arange(BS)
    n = np.arange(BS)
    C = np.zeros((BS, BS), dtype=np.float64)
    C[:, 0] = 1.0
    for s in range(1, 32):
        C[:, s] = np.cos(2 * np.pi * m * s / BS)
    C[:, 32] = np.cos(np.pi * m)
    for s in range(33, 64):
        C[:, s] = -np.sin(2 * np.pi * m * (s - 32) / BS)
    D = np.zeros((BS, BS), dtype=np.float64)
    D[0, :] = 1.0 / BS
    for s in range(1, 32):
        D[s, :] = (2.0 / BS) * np.cos(2 * np.pi * s * n / BS)
    D[32, :] = (1.0 / BS) * np.cos(np.pi * n)
    for s in range(33, 64):
        D[s, :] = -(2.0 / BS) * np.sin(2 * np.pi * (s - 32) * n / BS)
    C2 = np.kron(np.eye(2), C)   # [128, 128] blockdiag
    # D rows permuted to the interleaved (2f+ri) hc order produced by Sh2b:
    # row 2f -> hc slot f (cos part), row 2f+1 -> hc slot 32+f (sin part;
    # for f=0 that slot holds the Nyquist Yr_32 term).
    perm = np.empty(64, dtype=np.int64)
    perm[0::2] = np.arange(32)
    perm[1::2] = 32 + np.arange(32)
    D2 = np.kron(np.eye(2), D[perm, :])
    return C2, D2


def _wcat_matrix(weight):
    # einsum 'bjf,ijf->bif' => W_f[i, j] = rfft(weight, axis=-1)[i, j, f]
    wf = np.fft.rfft(weight.astype(np.float64), axis=-1)  # [i, j, F=33]
    Wcat = np.zeros((32, 128, 128), dtype=np.float64)
    Wcat[0, :64, :64] = wf[:, :, 0].real.T    # f=0 packed with f=32
    Wcat[0, 64:, 64:] = wf[:, :, 32].real.T
    for f in range(1, 32):
        Wr = wf[:, :, f].real.T               # [j, i]
        Wi = wf[:, :, f].imag.T
        Wcat[f, :64, :64] = Wr
        Wcat[f, 64:, :64] = -Wi
        Wcat[f, :64, 64:] = Wi
        Wcat[f, 64:, 64:] = Wr
    # Rows permuted to the interleaved (2j+ri) order produced by Sh1b's
    # stride-32 gather: new row 2j -> old jr row j, new row 2j+1 -> old ji
    # row 64+j.
    perm = np.empty(128, dtype=np.int64)
    perm[0::2] = np.arange(64)
    perm[1::2] = 64 + np.arange(64)
    Wcat = Wcat[:, perm, :]
    # Columns likewise interleaved to (2i+ri) so the S2 scatter-copy's
    # innermost out-dim is a contiguous (ri) pair.
    Wcat = Wcat[:, :, perm]
    # SBUF layout: [p, f*128 + c]
    return np.transpose(Wcat, (1, 0, 2)).reshape(128, 32 * 128)


# ---------------------------------------------------------------- bass program
def _build_nc(rows, dt_name):
    import concourse.bacc as bacc
    import concourse.mybir as mybir
    from concourse import tile

    DT = mybir.dt.float16 if dt_name == "fp16" else mybir.dt.float32
    FP32 = mybir.dt.float32
    esz = 2 if dt_name == "fp16" else 4
    TG = 2048 // (128 * esz)   # transposes per one-bank PSUM tile (8 fp16 / 4 fp32)
    NBT = rows // 128

    TDT = FP32 if REGT else DT          # transpose PSUM dtype
    TTG = 4 if REGT else TG             # transposes per one-bank PSUM tile
    nc = bacc.Bacc("TRN2", target_bir_lowering=False, debug=False,
                   num_devices=N_CORES,
                   name=f"bc_{dt_name}_{int(LDW_OPT)}_{int(SKEW)}_{int(REGT)}_{int(DMAT)}")

    def emit_transpose(nc_, out, in_, idt):
        if REGT:
            nc_.tensor.matmul(out, in_, idt)
        else:
            nc_.tensor.transpose(out, in_, idt)

    _copy_cnt = [0]

    def emit_copy(out, in_, fd=1024, fp16src=False):
        c = _copy_cnt[0]
        _copy_cnt[0] += 1
        # 17/32 of copies to DVE, 15/32 to ACT (ACT is ~10% slower per copy)
        if (c + 1) * 17 // 32 > c * 17 // 32:
            nc.vector.tensor_copy(out, in_)
        else:
            nc.scalar.copy(out, in_)

    NBTP = max(rows // 128, 1)
    x_t_d = nc.dram_tensor("x_t", [NBTP * IN_F, rows // NBTP],
                           DT, kind="ExternalInput").ap()
    c2_d = nc.dram_tensor("c2", [128, 128], DT, kind="ExternalInput").ap()
    wc_d = nc.dram_tensor("wcat", [128, 32 * 128], DT, kind="ExternalInput").ap()
    d2_d = nc.dram_tensor("d2", [128, 128], DT, kind="ExternalInput").ap()
    id_d = nc.dram_tensor("ident", [128, 128], DT, kind="ExternalInput").ap()
    y_d = nc.dram_tensor("y", [rows, IN_F], FP32, kind="ExternalOutput").ap()

    with tile.TileContext(nc) as tc:
        with (
            tc.tile_pool(name="consts", bufs=1) as cpool,
            tc.tile_pool(name="xt", bufs=1) as xt_pool,
            tc.tile_pool(name="work", bufs=2) as wpool,
            tc.tile_pool(name="yout", bufs=2) as ypool,
            tc.tile_pool(name="mmps", bufs=6, space="PSUM") as mmps,
            tc.tile_pool(name="tps", bufs=2, space="PSUM") as tps,
        ):
            c2 = cpool.tile([128, 128], DT, name="c2_sb")
            nc.sync.dma_start(c2[:], c2_d)
            wc = cpool.tile([128, 32 * 128], DT, name="wc_sb")
            nc.sync.dma_start(wc[:], wc_d)
            d2 = cpool.tile([128, 128], DT, name="d2_sb")
            nc.sync.dma_start(d2[:], d2_d)
            idt = cpool.tile([128, 128], DT, name="id_sb")
            nc.sync.dma_start(idt[:], id_d)

            bpw = rows // NBTP           # b-cols per bt-pair block
            xt = []                      # one tile per bt-pair: [128, 32*bpw]
            for bp in range(NBTP):
                t = xt_pool.tile([128, 32 * bpw], DT, name=f"xtp{bp}",
                                 tag="xtp", bufs=5)
                src = x_t_d[bp * IN_F:(bp + 1) * IN_F, :]
                src = src.rearrange("(jp p) c -> p jp c", p=128)
                nc.sync.dma_start(t[:], src)
                xt.append(t)

            def xt_slice(jp, bt):
                bp, loc = divmod(bt * 128, bpw)
                return xt[bp][:, jp * bpw + loc:jp * bpw + loc + 128]

            x1t_t = {}
            xcat_t = {}
            y2t_t = {}
            ycat_t = {}

            def emit_s1(bt):
                # S1: X1t[b, j*64+s] -- stationary = x_t tile, moving = C2
                x1t = wpool.tile([128, IN_F], DT, name="x1t", tag="x1t", bufs=3)
                for g in range(8):
                    ps = mmps.tile([128, 512], FP32, name="s1ps", tag="mmps")
                    for k in range(4):
                        jp = g * 4 + k
                        nc.tensor.matmul(ps[:, k * 128:(k + 1) * 128],
                                         xt_slice(jp, bt), c2[:])
                    emit_copy(x1t[:, g * 512:(g + 1) * 512], ps[:], fd=512)
                x1t_t[bt] = x1t

            def emit_sh1b(bt):
                # Sh1b: Xcat_f[2j+ri, b] = transpose of X1t cols {32k + f}
                # (col = j*64 + 32*ri + f = 32k + f, k = 2j+ri -- one stride).
                xcat = wpool.tile([128, IN_F], DT, name="xcat", tag="xcat", bufs=3)
                x1t_r = x1t_t.pop(bt).rearrange("p (k f) -> p f k", f=32)
                for g in range(32 // TTG):
                    tp = tps.tile([128, TTG * 128], TDT, name="t1ps", tag="tps")
                    for k in range(TTG):
                        f = g * TTG + k
                        emit_transpose(nc, tp[:, k * 128:(k + 1) * 128],
                                       x1t_r[:, f, :], idt[:])
                    emit_copy(xcat[:, g * TTG * 128:(g + 1) * TTG * 128],
                              tp[:], fd=TTG * 128, fp16src=not REGT)
                xcat_t[bt] = xcat

            def emit_s2(bt):
                # S2: stationary = Xcat_f, moving = Wcat_f; the PSUM->SBUF copy
                # scatters into i-major Y2t: col = i*64 + 2f + ri.
                xcat = xcat_t.pop(bt)
                y2t = wpool.tile([128, IN_F], DT, name="y2t", tag="y2t", bufs=3)
                y2t_r = y2t.rearrange("p (i f ri) -> p f i ri", f=32, ri=2)
                for g in range(8):
                    ps = mmps.tile([128, 512], FP32, name="s2ps", tag="mmps")
                    for k in range(4):
                        f = g * 4 + k
                        nc.tensor.matmul(ps[:, k * 128:(k + 1) * 128],
                                         xcat[:, f * 128:(f + 1) * 128],
                                         wc[:, f * 128:(f + 1) * 128])
                    emit_copy(y2t_r[:, g * 4:(g + 1) * 4, :, :], ps[:], fd=512)
                y2t_t[bt] = y2t

            def emit_sh2b(bt):
                # Sh2b: Ycat2_ip[2f+ri per i-half, b] = transpose of a
                # contiguous 128-col Y2t slice.
                y2t = y2t_t.pop(bt)
                ycat = wpool.tile([128, IN_F], DT, name="ycat", tag="ycat", bufs=3)
                if DMAT:
                    for ip in range(32):
                        nc.sync.dma_start(
                            ycat[:, ip * 128:(ip + 1) * 128],
                            y2t[:, ip * 128:(ip + 1) * 128],
                            transpose=True)
                    ycat_t[bt] = ycat
                    return
                for g in range(32 // TTG):
                    tp = tps.tile([128, TTG * 128], TDT, name="t2ps", tag="tps")
                    for k in range(TTG):
                        ip = g * TTG + k
                        emit_transpose(nc, tp[:, k * 128:(k + 1) * 128],
                                       y2t[:, ip * 128:(ip + 1) * 128],
                                       idt[:])
                    emit_copy(ycat[:, g * TTG * 128:(g + 1) * TTG * 128],
                              tp[:], fd=TTG * 128, fp16src=not REGT)
                ycat_t[bt] = ycat

            def emit_s3(bt):
                # S3: y[b, ip*128+nn] -- stationary = Ycat2_ip, moving = D2
                ycat = ycat_t.pop(bt)
                ysb = ypool.tile([128, IN_F], FP32, name="ysb", tag="ysb")
                for g in range(8):
                    ps = mmps.tile([128, 512], FP32, name="s3ps", tag="mmps")
                    for k in range(4):
                        ip = g * 4 + k
                        nc.tensor.matmul(ps[:, k * 128:(k + 1) * 128],
                                         ycat[:, ip * 128:(ip + 1) * 128],
                                         d2[:])
                    emit_copy(ysb[:, g * 512:(g + 1) * 512], ps[:], fd=512)
                    if g == 3:
                        nc.sync.dma_start(y_d[bt * 128:(bt + 1) * 128, 0:2048],
                                          ysb[:, 0:2048])
                nc.sync.dma_start(y_d[bt * 128:(bt + 1) * 128, 2048:4096],
                                  ysb[:, 2048:4096])

            stages = [emit_s1, emit_sh1b, emit_s2, emit_sh2b, emit_s3]
            if SKEW:
                # Software-pipelined emission: at step t, stage k works on
                # b-tile t-k, so matmul and transpose groups interleave.
                for step in range(NBT + len(stages) - 1):
                    for k, emit in enumerate(stages):
                        if 0 <= step - k < NBT:
                            emit(step - k)
            else:
                for bt in range(NBT):
                    for emit in stages:
                        emit(bt)

    nc.compile()
    return nc


def _get_nc(rows, dt_name):
    key = (rows, dt_name)
    if key not in _CACHE:
        _CACHE[key] = _build_nc(rows, dt_name)
    return _CACHE[key]


# ---------------------------------------------------------------- entry points
def run(x, weight, trace=False):
    if LDW_OPT:
        _patch_ldw_opt()
    from concourse.bass_utils import run_bass_kernel_spmd

    x = np.asarray(x, dtype=np.float32)
    weight = np.asarray(weight, dtype=np.float32)
    N = x.shape[0]
    rows = N // N_CORES
    np_dt = np.float16 if DT_NAME == "fp16" else np.float32

    C2, D2 = _hc_dft_matrices()
    consts = {
        "c2": C2.astype(np_dt),
        "wcat": _wcat_matrix(weight).astype(np_dt),
        "d2": D2.astype(np_dt),
        "ident": np.eye(128, dtype=np_dt),
    }
    x_t = np.ascontiguousarray(x.T).astype(np_dt)  # [4096, N]

    nbtp = max(rows // 128, 1)
    bpw = rows // nbtp
    in_maps = []
    for c in range(N_CORES):
        shard = x_t[:, c * rows:(c + 1) * rows]          # [4096, rows]
        blocks = shard.reshape(IN_F, nbtp, bpw).transpose(1, 0, 2)
        m = dict(consts)
        m["x_t"] = np.ascontiguousarray(blocks.reshape(nbtp * IN_F, bpw))
        in_maps.append(m)

    nc = _get_nc(rows, DT_NAME)
    res = run_bass_kernel_spmd(nc, in_maps, core_ids=list(range(N_CORES)),
                               trace=trace)
    y = np.concatenate([r["y"] for r in res.results], axis=0)
    return y, res


def kernel(x, weight):
    y, _ = run(x, weight)
    return y


# revision 34
# speedup vs baseline: 1.0310x; 1.0310x over previous
"""Block-circulant layer (p=q=64, bs=64) on 8 TRN2 NeuronCores.

y = x @ W_dense where W_dense is block-circulant, computed via the rfft
factorization as three dense matmul stages per core (data-parallel over rows):

  S1: per input-block pair jp: X1t[b, j*64+s] = x_t[j*64+m, b].T @ C2   (hc rfft)
  S2: per frequency f:        Y2t[b, f*128+iri] = Xcat_f.T @ Wcat_f    (complex mul)
  S3: per output-block pair:  y[b, ip*128+nn] = Ycat2_ip.T @ D2        (hc irfft)

"hc" = halfcomplex packing [Xr_0, Xr_1..31, Xr_32, Xi_1..31] -> exactly 64 real
slots per 64-wide block, so every stationary/moving operand is a clean 128x128.
Between stages, PE transposes (Sh1b/Sh2b) regroup j-major -> f-major -> i-major.
Stage outputs land directly in the layout the next stage consumes (the data is
the *stationary* operand of each matmul, so outputs come out batch-major).

Host side: x is transposed once (x_t [4096, 8192]) so no on-device input
transpose pass is needed; DFT/weight matrices are precomputed.
"""
import os
import numpy as np

N_CORES = 8
BS = 64
NBLK = 64              # p = q = 64 blocks
IN_F = NBLK * BS       # 4096
DT_NAME = os.environ.get("KERNEL_DTYPE", "fp16")
LDW_OPT = os.environ.get("KERNEL_LDW_OPT", "0") == "1"
SKEW = os.environ.get("KERNEL_SKEW", "1") == "1"
# Transposes as regular-mode matmuls (counts as PE-busy for HAM, so the
# 2.4 GHz clock stays engaged; costs the same at 16-bit dtypes).
REGT = os.environ.get("KERNEL_REGT", "0") == "1"
# Sh2b via DMA xbar transpose instead of the tensor engine.
DMAT = os.environ.get("KERNEL_DMAT", "0") == "1"

_CACHE = {}


def _patch_ldw_opt():
    """Flip walrus's --enable-ldw-opt to true for this process."""
    import concourse.bass_utils as bu

    if getattr(bu, "_ldw_patched", False):
        return
    orig = bu.run_command

    def patched(argv, **kw):
        argv = ["--enable-ldw-opt=true" if a == "--enable-ldw-opt=false" else a
                for a in argv]
        return orig(argv, **kw)

    bu.run_command = patched
    bu._ldw_patched = True


# ---------------------------------------------------------------- host matrices
def _hc_dft_matrices():
    m = np.arange(BS)
    n = np.arange(BS)
    C = np.zeros((BS, BS), dtype=np.float64)
    C[:, 0] = 1.0
    for s in range(1, 32):
        C[:, s] = np.cos(2 * np.pi * m * s / BS)
    C[:, 32] = np.cos(np.pi * m)
    for s in range(33, 64):
        C[:, s] = -np.sin(2 * np.pi * m * (s - 32) / BS)
    D = np.zeros((BS, BS), dtype=np.float64)
    D[0, :] = 1.0 / BS
    for s in range(1, 32):
        D[s, :] = (2.0 / BS) * np.cos(2 * np.pi * s * n / BS)
    D[32, :] = (1.0 / BS) * np.cos(np.pi * n)
    for s in range(33, 64):
        D[s, :] = -(2.0 / BS) * np.sin(2 * np.pi * (s - 32) * n / BS)
    C2 = np.kron(np.eye(2), C)   # [128, 128] blockdiag
    # D rows permuted to the interleaved (2f+ri) hc order produced by Sh2b:
    # row 2f -> hc slot f (cos part), row 2f+1 -> hc slot 32+f (sin part;
    # for f=0 that slot holds the Nyquist Yr_32 term).
    perm = np.empty(64, dtype=np.int64)
    perm[0::2] = np.arange(32)
    perm[1::2] = 32 + np.arange(32)
    D2 = np.kron(np.eye(2), D[perm, :])
    return C2, D2


def _wcat_matrix(weight):
    # einsum 'bjf,ijf->bif' => W_f[i, j] = rfft(weight, axis=-1)[i, j, f]
    wf = np.fft.rfft(weight.astype(np.float64), axis=-1)  # [i, j, F=33]
    Wcat = np.zeros((32, 128, 128), dtype=np.float64)
    Wcat[0, :64, :64] = wf[:, :, 0].real.T    # f=0 packed with f=32
    Wcat[0, 64:, 64:] = wf[:, :, 32].real.T
    for f in range(1, 32):
        Wr = wf[:, :, f].real.T               # [j, i]
        Wi = wf[:, :, f].imag.T
        Wcat[f, :64, :64] = Wr
        Wcat[f, 64:, :64] = -Wi
        Wcat[f, :64, 64:] = Wi
        Wcat[f, 64:, 64:] = Wr
    # Rows permuted to the interleaved (2j+ri) order produced by Sh1b's
    # stride-32 gather: new row 2j -> old jr row j, new row 2j+1 -> old ji
    # row 64+j.
    perm = np.empty(128, dtype=np.int64)
    perm[0::2] = np.arange(64)
    perm[1::2] = 64 + np.arange(64)
    Wcat = Wcat[:, perm, :]
    # Columns likewise interleaved to (2i+ri) so the S2 scatter-copy's
    # innermost out-dim is a contiguous (ri) pair.
    Wcat = Wcat[:, :, perm]
    # SBUF layout: [p, f*128 + c]
    return np.transpose(Wcat, (1, 0, 2)).reshape(128, 32 * 128)


# ---------------------------------------------------------------- bass program
def _build_nc(rows, dt_name):
    import concourse.bacc as bacc
    import concourse.mybir as mybir
    from concourse import tile

    DT = mybir.dt.float16 if dt_name == "fp16" else mybir.dt.float32
    FP32 = mybir.dt.float32
    esz = 2 if dt_name == "fp16" else 4
    TG = 2048 // (128 * esz)   # transposes per one-bank PSUM tile (8 fp16 / 4 fp32)
    NBT = rows // 128

    TDT = FP32 if REGT else DT          # transpose PSUM dtype
    TTG = 4 if REGT else TG             # transposes per one-bank PSUM tile
    nc = bacc.Bacc("TRN2", target_bir_lowering=False, debug=False,
                   num_devices=N_CORES,
                   name=f"bc_{dt_name}_{int(LDW_OPT)}_{int(SKEW)}_{int(REGT)}_{int(DMAT)}")

    def emit_transpose(nc_, out, in_, idt):
        if REGT:
            nc_.tensor.matmul(out, in_, idt)
        else:
            nc_.tensor.transpose(out, in_, idt)

    _copy_cnt = [0]

    def emit_copy(out, in_, fd=1024, fp16src=False):
        c = _copy_cnt[0]
        _copy_cnt[0] += 1
        # 17/32 of copies to DVE, 15/32 to ACT (ACT is ~10% slower per copy)
        if (c + 1) * 17 // 32 > c * 17 // 32:
            nc.vector.tensor_copy(out, in_)
        else:
            nc.scalar.copy(out, in_)

    NBTP = max(rows // 128, 1)
    x_t_d = nc.dram_tensor("x_t", [NBTP * IN_F, rows // NBTP],
                           DT, kind="ExternalInput").ap()
    c2_d = nc.dram_tensor("c2", [128, 128], DT, kind="ExternalInput").ap()
    wc_d = nc.dram_tensor("wcat", [128, 32 * 128], DT, kind="ExternalInput").ap()
    d2_d = nc.dram_tensor("d2", [128, 128], DT, kind="ExternalInput").ap()
    id_d = nc.dram_tensor("ident", [128, 128], DT, kind="ExternalInput").ap()
    y_d = nc.dram_tensor("y", [rows, IN_F], FP32, kind="ExternalOutput").ap()

    with tile.TileContext(nc) as tc:
        with (
            tc.tile_pool(name="consts", bufs=1) as cpool,
            tc.tile_pool(name="xt", bufs=1) as xt_pool,
            tc.tile_pool(name="work", bufs=2) as wpool,
            tc.tile_pool(name="yout", bufs=2) as ypool,
            tc.tile_pool(name="mmps", bufs=6, space="PSUM") as mmps,
            tc.tile_pool(name="tps", bufs=2, space="PSUM") as tps,
        ):
            # Tiny consts S1 needs immediately, then input block 0, then the
            # fat constants (wcat is 1 MB but only needed at skew step 2).
            c2 = cpool.tile([128, 128], DT, name="c2_sb")
            nc.sync.dma_start(c2[:], c2_d)
            idt = cpool.tile([128, 128], DT, name="id_sb")
            nc.sync.dma_start(idt[:], id_d)

            bpw = rows // NBTP           # b-cols per block
            xt = []                      # one tile per block: [128, 32*bpw]
            for bp in range(NBTP):
                t = xt_pool.tile([128, 32 * bpw], DT, name=f"xtp{bp}")
                xt.append(t)

            def load_block(bp):
                src = x_t_d[bp * IN_F:(bp + 1) * IN_F, :]
                src = src.rearrange("(jp p) c -> p jp c", p=128)
                nc.sync.dma_start(xt[bp][:], src)

            load_block(0)
            wc = cpool.tile([128, 32 * 128], DT, name="wc_sb")
            nc.sync.dma_start(wc[:], wc_d)
            if NBTP > 1:
                load_block(1)
            d2 = cpool.tile([128, 128], DT, name="d2_sb")
            nc.sync.dma_start(d2[:], d2_d)
            for bp in range(2, NBTP):
                load_block(bp)

            def xt_slice(jp, bt):
                bp, loc = divmod(bt * 128, bpw)
                return xt[bp][:, jp * bpw + loc:jp * bpw + loc + 128]

            x1t_t = {}
            xcat_t = {}
            y2t_t = {}
            ycat_t = {}

            def emit_s1(bt):
                # S1: X1t[b, j*64+s] -- stationary = x_t tile, moving = C2
                x1t = wpool.tile([128, IN_F], DT, name="x1t", tag="x1t", bufs=3)
                for g in range(8):
                    ps = mmps.tile([128, 512], FP32, name="s1ps", tag="mmps")
                    for k in range(4):
                        jp = g * 4 + k
                        nc.tensor.matmul(ps[:, k * 128:(k + 1) * 128],
                                         xt_slice(jp, bt), c2[:])
                    emit_copy(x1t[:, g * 512:(g + 1) * 512], ps[:], fd=512)
                x1t_t[bt] = x1t

            def emit_sh1b(bt):
                # Sh1b: Xcat_f[2j+ri, b] = transpose of X1t cols {32k + f}
                # (col = j*64 + 32*ri + f = 32k + f, k = 2j+ri -- one stride).
                xcat = wpool.tile([128, IN_F], DT, name="xcat", tag="xcat", bufs=3)
                x1t_r = x1t_t.pop(bt).rearrange("p (k f) -> p f k", f=32)
                for g in range(32 // TTG):
                    tp = tps.tile([128, TTG * 128], TDT, name="t1ps", tag="tps")
                    for k in range(TTG):
                        f = g * TTG + k
                        emit_transpose(nc, tp[:, k * 128:(k + 1) * 128],
                                       x1t_r[:, f, :], idt[:])
                    emit_copy(xcat[:, g * TTG * 128:(g + 1) * TTG * 128],
                              tp[:], fd=TTG * 128, fp16src=not REGT)
                xcat_t[bt] = xcat

            def emit_s2(bt):
                # S2: stationary = Xcat_f, moving = Wcat_f; the PSUM->SBUF copy
                # scatters into i-major Y2t: col = i*64 + 2f + ri.
                xcat = xcat_t.pop(bt)
                y2t = wpool.tile([128, IN_F], DT, name="y2t", tag="y2t")
                y2t_r = y2t.rearrange("p (i f ri) -> p f i ri", f=32, ri=2)
                for g in range(8):
                    ps = mmps.tile([128, 512], FP32, name="s2ps", tag="mmps")
                    for k in range(4):
                        f = g * 4 + k
                        nc.tensor.matmul(ps[:, k * 128:(k + 1) * 128],
                                         xcat[:, f * 128:(f + 1) * 128],
                                         wc[:, f * 128:(f + 1) * 128])
                    emit_copy(y2t_r[:, g * 4:(g + 1) * 4, :, :], ps[:], fd=512)
                y2t_t[bt] = y2t

            def emit_sh2b(bt):
                # Sh2b: Ycat2_ip[2f+ri per i-half, b] = transpose of a
                # contiguous 128-col Y2t slice.
                y2t = y2t_t.pop(bt)
                ycat = wpool.tile([128, IN_F], DT, name="ycat", tag="ycat")
                if DMAT:
                    for ip in range(32):
                        nc.sync.dma_start(
                            ycat[:, ip * 128:(ip + 1) * 128],
                            y2t[:, ip * 128:(ip + 1) * 128],
                            transpose=True)
                    ycat_t[bt] = ycat
                    return
                for g in range(32 // TTG):
                    tp = tps.tile([128, TTG * 128], TDT, name="t2ps", tag="tps")
                    for k in range(TTG):
                        ip = g * TTG + k
                        emit_transpose(nc, tp[:, k * 128:(k + 1) * 128],
                                       y2t[:, ip * 128:(ip + 1) * 128],
                                       idt[:])
                    emit_copy(ycat[:, g * TTG * 128:(g + 1) * TTG * 128],
                              tp[:], fd=TTG * 128, fp16src=not REGT)
                ycat_t[bt] = ycat

            def emit_s3(bt):
                # S3: y[b, ip*128+nn] -- stationary = Ycat2_ip, moving = D2
                ycat = ycat_t.pop(bt)
                ysb = ypool.tile([128, IN_F], FP32, name="ysb", tag="ysb")
                for g in range(8):
                    ps = mmps.tile([128, 512], FP32, name="s3ps", tag="mmps")
                    for k in range(4):
                        ip = g * 4 + k
                        nc.tensor.matmul(ps[:, k * 128:(k + 1) * 128],
                                         ycat[:, ip * 128:(ip + 1) * 128],
                                         d2[:])
                    emit_copy(ysb[:, g * 512:(g + 1) * 512], ps[:], fd=512)
                    if g == 3:
                        nc.sync.dma_start(y_d[bt * 128:(bt + 1) * 128, 0:2048],
                                          ysb[:, 0:2048])
                nc.sync.dma_start(y_d[bt * 128:(bt + 1) * 128, 2048:4096],
                                  ysb[:, 2048:4096])

            stages = [emit_s1, emit_sh1b, emit_s2, emit_sh2b, emit_s3]
            if SKEW:
                # Software-pipelined emission: at step t, stage k works on
                # b-tile t-k, so matmul and transpose groups interleave.
                for step in range(NBT + len(stages) - 1):
                    for k, emit in enumerate(stages):
                        if 0 <= step - k < NBT:
                            emit(step - k)
            else:
                for bt in range(NBT):
                    for emit in stages:
                        emit(bt)

    nc.compile()
    return nc


def _get_nc(rows, dt_name):
    key = (rows, dt_name)
    if key not in _CACHE:
        _CACHE[key] = _build_nc(rows, dt_name)
    return _CACHE[key]


# ---------------------------------------------------------------- entry points
def run(x, weight, trace=False):
    if LDW_OPT:
        _patch_ldw_opt()
    from concourse.bass_utils import run_bass_kernel_spmd

    x = np.asarray(x, dtype=np.float32)
    weight = np.asarray(weight, dtype=np.float32)
    N = x.shape[0]
    rows = N // N_CORES
    np_dt = np.float16 if DT_NAME == "fp16" else np.float32

    C2, D2 = _hc_dft_matrices()
    consts = {
        "c2": C2.astype(np_dt),
        "wcat": _wcat_matrix(weight).astype(np_dt),
        "d2": D2.astype(np_dt),
        "ident": np.eye(128, dtype=np_dt),
    }
    x_t = np.ascontiguousarray(x.T).astype(np_dt)  # [4096, N]

    nbtp = max(rows // 128, 1)
    bpw = rows // nbtp
    in_maps = []
    for c in range(N_CORES):
        shard = x_t[:, c * rows:(c + 1) * rows]          # [4096, rows]
        blocks = shard.reshape(IN_F, nbtp, bpw).transpose(1, 0, 2)
        m = dict(consts)
        m["x_t"] = np.ascontiguousarray(blocks.reshape(nbtp * IN_F, bpw))
        in_maps.append(m)

    nc = _get_nc(rows, DT_NAME)
    res = run_bass_kernel_spmd(nc, in_maps, core_ids=list(range(N_CORES)),
                               trace=trace)
    y = np.concatenate([r["y"] for r in res.results], axis=0)
    return y, res


def kernel(x, weight):
    y, _ = run(x, weight)
    return y
